# revision 37
# baseline (speedup 1.0000x reference)
"""CIN (Compressed Interaction Network) kernel for Trainium2, 8 NeuronCores.

Reference computation (per sample b, NFIELD=64, NEMB=64, NFILTER=128, 3 layers):
    xk_{l+1}[o, e] = relu( sum_{f,c} W_l[o, f*C+c] * x0[f, e] * xk_l[c, e] )
    pooled_l = sum_e xk_{l+1};  y = concat(pooled) @ Wa.T

Strategy:
  - Data-parallel over batch: 32 samples/core, free axis J = 32*64 = 2048 (b-major,
    e-minor). Columns are independent through all layers.
  - Per layer the GEMM is out = W @ H with H[(f,c), j] = x0[f,j] * xk[c,j]
    (Khatri-Rao column structure). H is materialized K-tile by K-tile in bf16 by
    DVE tensor_tensor with plain 2D unit-stride APs (DVE 2x_1P perf mode).
  - Free blocks are processed in PAIRS with the K loop outermost inside a
    layer, so each stationary weight tile serves two matmuls into two PSUM
    banks: a full-array LDWEIGHTS cannot overlap an in-flight matmul's
    drain, so single-use weights pace at (398+N)/2.4 = 379 ns; two fills
    per load amortize the reload bubble.
  - Layer 0 is symmetric (xk = x0): W0 is host-folded onto upper-triangle
    (f<=c) pairs, K = 2080 -> 17 K-tiles, with both TT operands host-gathered
    (x0packf/x0packc) and DMA'd straight from DRAM.
  - Layers 1-2 modulator rows are partition-replicated in "oct" tiles of
    8 fields x one free block: one 32-partition seed DMA from a host-side
    replicated array + 2 partition-doubling SBUF->SBUF DMAs, issued
    chain-major to avoid Sync-queue head-of-line blocking.
  - ScalarE applies ReLU into a 4x-repeated next-layer input xk4; VectorE
    reduces pooled sums in fp32; a tiny fp32 head matmul folds Wa.
"""

import sys

if "/opt/trn_rl_repo" not in sys.path:
    sys.path.insert(0, "/opt/trn_rl_repo")

import numpy as np
import ml_dtypes

B, F, E, O = 256, 64, 64, 128
NCORES = 8
BC = B // NCORES          # samples per core
J = BC * E                # free columns per core
JB = 512                  # free-block size (one PSUM bank)
NJ = J // JB              # 4 free blocks (2 pairs)
KT0 = 17                  # layer-0 K-tiles (packed symmetric, 2176 = 17*128)
K0 = KT0 * 128
KT = [KT0, 64, 64]

_BF16 = ml_dtypes.bfloat16
_STATE = {}

_PAIRS = [(f, c) for f in range(F) for c in range(f, F)]
_F_IDX = np.array([p[0] for p in _PAIRS] + [0] * (K0 - len(_PAIRS)), np.int64)
_C_IDX = np.array([p[1] for p in _PAIRS] + [0] * (K0 - len(_PAIRS)), np.int64)


def _build_nc():
    import concourse.bass as bass
    import concourse.tile as tile
    import concourse.mybir as mybir
    from concourse import bacc

    dt = mybir.dt
    nc = bacc.Bacc("TRN2", target_bir_lowering=False, debug=False)

    # seeds for oct modulator tiles: [NJ*8 octs, 32 partitions, 8*JB]
    x0seed = nc.dram_tensor(
        "x0seed", [NJ * 8, 32, 8 * JB], dt.bfloat16, kind="ExternalInput"
    )
    x0packf = nc.dram_tensor(
        "x0packf", [NJ, 128, KT0 * JB], dt.bfloat16, kind="ExternalInput"
    )
    x0packc = nc.dram_tensor(
        "x0packc", [NJ, 128, KT0 * JB], dt.bfloat16, kind="ExternalInput"
    )
    w0t = nc.dram_tensor("w0t", [128, KT0 * O], dt.bfloat16, kind="ExternalInput")
    w1t = nc.dram_tensor("w1t", [128, 64 * O], dt.bfloat16, kind="ExternalInput")
    w2t = nc.dram_tensor("w2t", [128, 64 * O], dt.bfloat16, kind="ExternalInput")
    wa = nc.dram_tensor("wa", [O, 3], dt.float32, kind="ExternalInput")
    y = nc.dram_tensor("y", [1, BC], dt.float32, kind="ExternalOutput")

    OCTW = 8 * JB             # free width of an 8-field modulator tile
    PKW = KT0 * JB            # free width of a packed layer-0 operand tile

    with tile.TileContext(nc) as tc:
        with (
            tc.tile_pool(name="wpool", bufs=1) as wpool,
            tc.tile_pool(name="xpool", bufs=1) as xpool,
            tc.tile_pool(name="modpool", bufs=8) as modpool,
            tc.tile_pool(name="packpool", bufs=1) as packpool,
            tc.tile_pool(name="hpool", bufs=6) as hpool,
            tc.tile_pool(name="xkpool", bufs=5) as xkpool,
            tc.tile_pool(name="psum", bufs=4, space="PSUM") as psum_pool,
            tc.tile_pool(name="psumy", bufs=1, space="PSUM") as psumy_pool,
        ):
            # --- static loads -------------------------------------------------
            wa_sb = xpool.tile([O, 3], dt.float32, tag="wa")
            nc.sync.dma_start(wa_sb[:], wa[:])
            w_sb = []
            for li, (wd, kt) in enumerate(zip((w0t, w1t, w2t), KT)):
                w = wpool.tile([128, kt, O], dt.bfloat16, tag=f"w{li}", name=f"w{li}")
                nc.sync.dma_start(w[:].rearrange("p t o -> p (t o)"), wd[:])
                w_sb.append(w)
            pooled = [
                xpool.tile([O, BC], dt.float32, tag=f"pooled{l}", name=f"pooled{l}")
                for l in range(3)
            ]

            def mod_oct(oj):
                """Build modulator oct tile for global oct index oj
                (oj = 8*jj + o): m[p, 512*i+e] = x0[8*o+i, jj-block][e]."""
                m = modpool.tile([128, OCTW], dt.bfloat16, tag="mod", name=f"m{oj}")
                nc.sync.dma_start(m[0:32, :], x0seed[oj])
                nc.sync.dma_start(m[32:64, :], m[0:32, :])
                nc.sync.dma_start(m[64:128, :], m[0:64, :])
                return m

            # --- main loop over free-block pairs ------------------------------
            for jp in range(2):
                js = (2 * jp, 2 * jp + 1)
                # layer-0 packed operands + modulator octs for this pair
                p0 = {}
                for j in js:
                    p0f = packpool.tile(
                        [128, PKW], dt.bfloat16, tag="p0f", name=f"p0f{j}"
                    )
                    p0c = packpool.tile(
                        [128, PKW], dt.bfloat16, tag="p0c", name=f"p0c{j}"
                    )
                    nc.sync.dma_start(p0f[:], x0packf[j])
                    nc.sync.dma_start(p0c[:], x0packc[j])
                    p0[j] = (p0f, p0c)
                mods = {}

                # ---- layer 0 (j-major; single-use weights) -------------------
                accs = {}
                for j in js:
                    acc = psum_pool.tile(
                        [128, JB], dt.float32, tag="acc", name=f"acc0_{j}"
                    )
                    p0f, p0c = p0[j]
                    for s in range(9):
                        nk = 2 if s < 8 else 1
                        h = hpool.tile(
                            [128, 4 * JB], dt.bfloat16, tag="h", name=f"h0_{j}_{s}"
                        )
                        w_ = JB * nk
                        nc.vector.tensor_tensor(
                            h[:, 0:w_],
                            p0c[:, 2 * JB * s : 2 * JB * s + w_],
                            p0f[:, 2 * JB * s : 2 * JB * s + w_],
                            op=mybir.AluOpType.mult,
                        )
                        for i in range(nk):
                            t = 2 * s + i
                            nc.tensor.matmul(
                                acc[:], w_sb[0][:, t, :],
                                h[:, JB * i : JB * (i + 1)],
                                start=(t == 0), stop=(t == KT0 - 1),
                            )
                    accs[j] = acc

                xk4 = {}
                for l in range(1, 3):
                    # epilogue of previous layer: relu 4x -> xk4, pooled reduce
                    for j in js:
                        xk4_new = xkpool.tile(
                            [128, 4 * JB], dt.bfloat16, tag="xk4", name=f"xk4_{l}_{j}"
                        )
                        for i in range(4):
                            nc.scalar.activation(
                                xk4_new[:, JB * i : JB * (i + 1)], accs[j][:],
                                mybir.ActivationFunctionType.Relu,
                            )
                        nc.vector.tensor_reduce(
                            pooled[l - 1][:, 8 * j : 8 * j + 8],
                            xk4_new[:, 0:JB].rearrange("p (b e) -> p b e", e=E),
                            axis=mybir.AxisListType.X,
                            op=mybir.AluOpType.add,
                        )
                        xk4[j] = xk4_new
                    # ---- layer l: K-major over the pair (2x weight reuse) ----
                    accs = {}
                    for j in js:
                        accs[j] = psum_pool.tile(
                            [128, JB], dt.float32, tag="acc", name=f"acc{l}_{j}"
                        )
                    # L1 sweeps octs 0..7 (building them); L2 sweeps 7..0 so
                    # the octs still resident in the pool are reused and only
                    # the aged-out ones are rebuilt.
                    sweep = list(range(8)) if l == 1 else list(range(7, -1, -1))
                    for o in sweep:
                        for j in js:
                            if (o, j) not in mods or (l == 2 and o < 4):
                                mods[(o, j)] = mod_oct(8 * j + o)
                        for s in range(2):
                            hq = {}
                            for j in js:
                                h = hpool.tile(
                                    [128, 4 * JB], dt.bfloat16, tag="h",
                                    name=f"h{l}_{j}_{o}_{s}",
                                )
                                if o == sweep[0] and s == 0:
                                    # the layer's first quad reads only the
                                    # first ReLU slice so it starts early
                                    for i in range(4):
                                        nc.vector.tensor_tensor(
                                            h[:, JB * i : JB * (i + 1)],
                                            xk4[j][:, 0:JB],
                                            mods[(o, j)][:, JB * i : JB * (i + 1)],
                                            op=mybir.AluOpType.mult,
                                        )
                                else:
                                    nc.vector.tensor_tensor(
                                        h[:], xk4[j][:],
                                        mods[(o, j)][
                                            :, 4 * JB * s : 4 * JB * (s + 1)
                                        ],
                                        op=mybir.AluOpType.mult,
                                    )
                                hq[j] = h
                            for i in range(4):
                                t = 8 * o + 4 * s + i
                                for j in js:
                                    nc.tensor.matmul(
                                        accs[j][:], w_sb[l][:, t, :],
                                        hq[j][:, JB * i : JB * (i + 1)],
                                        start=(o == sweep[0] and s == 0 and i == 0),
                                        stop=(o == sweep[-1] and s == 1 and i == 3),
                                    )
                # final epilogue (layer 2): only pooled is needed
                for j in js:
                    xk_last = xkpool.tile(
                        [128, JB], dt.bfloat16, tag="xklast", name=f"xkl_{j}"
                    )
                    nc.scalar.activation(
                        xk_last[:], accs[j][:], mybir.ActivationFunctionType.Relu
                    )
                    nc.vector.tensor_reduce(
                        pooled[2][:, 8 * j : 8 * j + 8],
                        xk_last[:].rearrange("p (b e) -> p b e", e=E),
                        axis=mybir.AxisListType.X,
                        op=mybir.AluOpType.add,
                    )

            # --- head: y[b] = sum_l wa[:, l] . pooled[l][:, b] ----------------
            yac = psumy_pool.tile([1, BC], dt.float32, tag="yac")
            for l in range(3):
                nc.tensor.matmul(
                    yac[:], wa_sb[:, l : l + 1], pooled[l][:],
                    start=(l == 0), stop=(l == 2),
                )
            y_sb = xpool.tile([1, BC], dt.float32, tag="ysb")
            nc.scalar.copy(y_sb[:], yac[:])
            nc.sync.dma_start(y[:], y_sb[:])

    nc.finalize()
    return nc


def _get_nc():
    if "nc" not in _STATE:
        _STATE["nc"] = _build_nc()
    return _STATE["nc"]


def _pack_w0(W0):
    # fold symmetric (f, c) weight pairs onto f <= c; pad to K0 with zeros
    w = np.asarray(W0, np.float32).reshape(O, F, F)
    wp = np.zeros((O, K0), np.float32)
    k = 0
    for f in range(F):
        wp[:, k] = w[:, f, f]
        k += 1
        n = F - f - 1
        if n:
            wp[:, k : k + n] = w[:, f, f + 1 :] + w[:, f + 1 :, f]
            k += n
    return wp


def _prep_in_maps(x, W0, W1, W2, Wa):
    x = np.asarray(x, dtype=np.float32)

    def w_layout(wt):
        K = wt.shape[0]
        return np.ascontiguousarray(
            wt.reshape(K // 128, 128, O).transpose(1, 0, 2).reshape(128, -1)
        )

    w0t = w_layout(_pack_w0(W0).T).astype(_BF16)
    w1t = w_layout(np.ascontiguousarray(np.asarray(W1, np.float32).T)).astype(_BF16)
    w2t = w_layout(np.ascontiguousarray(np.asarray(W2, np.float32).T)).astype(_BF16)
    wa = np.ascontiguousarray(np.asarray(Wa, np.float32).reshape(3, O).T)

    def pack_gather(x0b, idx):
        g = x0b[idx]                                        # (K0, J)
        g = g.reshape(KT0, 128, NJ, JB).transpose(2, 1, 0, 3)
        return np.ascontiguousarray(g.reshape(NJ, 128, KT0 * JB))

    in_maps = []
    for c in range(NCORES):
        xc = x[c * BC : (c + 1) * BC]                       # (BC, F, E)
        x0 = np.ascontiguousarray(xc.transpose(1, 0, 2).reshape(F, J))
        x0b = x0.astype(_BF16)
        x0r = x0b.reshape(F, NJ, JB)
        seeds = np.empty((NJ * 8, 32, 8 * JB), _BF16)
        for jj in range(NJ):
            for o in range(8):
                blk = x0r[8 * o : 8 * o + 8, jj].reshape(1, 8 * JB)
                seeds[8 * jj + o] = np.broadcast_to(blk, (32, 8 * JB))
        in_maps.append(
            {
                "x0seed": seeds,
                "x0packf": pack_gather(x0b, _F_IDX),
                "x0packc": pack_gather(x0b, _C_IDX),
                "w0t": w0t,
                "w1t": w1t,
                "w2t": w2t,
                "wa": wa,
            }
        )
    return in_maps


def _run(inputs, trace=False, **kwargs):
    from concourse.bass_utils import run_bass_kernel_spmd

    nc = _get_nc()
    in_maps = _prep_in_maps(**inputs)
    res = run_bass_kernel_spmd(
        nc, in_maps, core_ids=list(range(NCORES)), trace=trace, **kwargs
    )
    y = np.concatenate(
        [np.asarray(r["y"], np.float32).reshape(BC) for r in res.results]
    )
    return y, res


def kernel(**inputs) -> np.ndarray:
    y, _ = _run(inputs, trace=False)
    return y
